# revision 9
# baseline (speedup 1.0000x reference)
"""Trainium2 Bass kernel for nn_DirectEncodingModel (gnn_message_passing).

Model (reference):
    h = x                                  # [B, 256]
    for l in 0..2:
        gathered = h[:, idx[l]]            # [B, 4, 128]
        z = einsum('bgk,gku->bgu', gathered, W[l]) + b[l]
        h = tanh(z).reshape(B, 256)
    out = h @ W_out + b_out                # [B, 10]

Host-side transforms (exact):
  * levels 1-2: fold the gather into dense per-level weights
        Weff[l][d, g*64+u] = sum_{k: idx[l,g,k]==d} W[l,g,k,u]
    so each level is h = tanh(h @ Weff[l]) — a dense [B,256]@[256,256]
    matmul.
  * level 0: host pre-gathers x per group (xg[g] = x[:, idx[0,g]]); the
    device runs one K=128, M=64 matmul per group with raw W[0,g]; the two
    M=64 halves of a pair occupy distinct PE column groups
    (tile_position via base partitions) and stream concurrently.

Device layout: activations transposed — [feature(partition), batch(free)],
fp16 matmuls with fp32 PSUM accumulation.

The bottleneck is the tanh: 12.58M elements/core, and the scalar (ACT)
engine alone runs ~96 us at 1 elem/lane/cycle.  v2 splits the tanh
between the ACT engine (table tanh) and the Vector engine (DVE) using two
runtime-registered custom DVE ops that evaluate a clamped degree-9 odd
minimax polynomial (max |err| vs tanh = 7.6e-3; end-to-end rel err
~8e-3 with the rotation schedule below, tolerance 2e-2):

    TANH9A: h1 = ((a4*t + a3)*t + a2)*t,        t = min(z^2, c^2)
    TANH9B: y  = clip(((h1 + a1)*t + a0)*z, -1, 1)

Each custom op processes 1 elem/lane/cycle on the DVE (multi-uop single
instruction), so a DVE tanh pair costs ~2.3 us/level-chunk vs ACT 0.69
(both HW-measured) — the balanced split is 5/8 of chunks' rotating level
on the DVE (see _dve_level).  The PE (fp16 matmul measured at ~0.21
ns/col, 2x the cost model; L0 uses M=64 column-pair concurrency) runs
~47 us, under the ~53 us tanh-engine wall.

Out layer: 3 chunks' [10,512] results pack into one PSUM tile at
partitions {0,32,64} via tile_position, one DVE copy per 3 chunks, DMA
out from SBUF on the gpsimd ring.

x DMA (16.8 MB/core fp16, pre-gathered) alternates between the sync and
gpsimd DGE rings — a single ring was rate-limiting (~19 us of exposed
DMA in the marginal-rep measurement) — with a 5-block prefetch horizon.
Per tick the DVE-owned level's matmuls are emitted first so the DVE feed
never queues behind PE write-after-read stalls on ACT-owned levels.

Software pipeline (skewed emission out(i-3) | L2(i-2) | L1(i-1) | L0(i))
is unchanged from v1.  Measured: ~62 us/rep marginal (paired-slope, 150
pairs), rel err 7.4e-3; v1 baseline was ~91 us.  Sharding: pure batch
data parallelism across 8 cores; weights replicated.
"""

import numpy as np

import concourse.mybir as mybir
import concourse.bacc as bacc
import concourse.tile as tile
from concourse.bass_utils import run_bass_kernel_spmd

F16 = mybir.dt.float16
F32 = mybir.dt.float32

N_CORES = 8
B, D, L, G, K, U, OUT = 131072, 256, 3, 4, 128, 64, 10
GU = G * U  # 256
BS = B // N_CORES  # 16384 per core

CHUNK = 512           # batch columns per level-computation (one PSUM slot)
NCHUNK = BS // CHUNK  # 32
XBLK = 1024           # batch columns per x DMA
OBLK = 1024           # batch columns per output DMA

# test-harness hooks (harness never touches these; defaults are production)
TRACE = False
LAST_RESULTS = None

_PROG_CACHE = {}

# ---------------------------------------------------------------------------
# Custom DVE tanh (degree-9 odd minimax, ramp + output clamp form)
# y = clip(z*(a0 + t*(a1 + t*(a2 + t*(a3 + t*a4)))), -1, 1), t = min(z^2, c^2)
P9_A0 = 0.9800317416370952
P9_A1 = -0.26106888154251784
P9_A2 = 0.05544444997442462
P9_A3 = -0.006477015514642958
P9_A4 = 0.0003014239240568509
P9_C = 2.969877822220705
P9_CSQ = P9_C * P9_C

_TANH9 = None  # (opA, opB) after registration


def _register_tanh9():
    """Register the two custom DVE ops with concourse.dve_ops at runtime
    (the documented extension point is the module-level OPS list; we can't
    edit the read-only repo, so we append in-process before any compile)."""
    global _TANH9
    if _TANH9 is not None:
        return _TANH9
    import concourse.dve_ops as dve_ops
    from concourse.dve_spec import (Spec, Src0, Src1, C0, C1, C2, C3, Zero,
                                    One, maxx, minn, sq, lower,
                                    _spill_c3_to_src1, _has_src1)
    from concourse.dve_uop import DveOpSpec

    # A: h1 = ((s1*t + imm2)*t + <in1:[P,1]>)*t, t = min(z^2, s0)
    _t = minn(sq(Src0), C0)
    bodyA = _spill_c3_to_src1(((C1 * _t + C2) * _t + C3) * _t)

    def refA(in0, in1, s0, s1, imm2):
        t = np.minimum(in0.astype(np.float32) ** 2, np.float32(s0))
        r = ((np.float32(s1) * t + np.float32(imm2)) * t
             + np.asarray(in1, np.float32)) * t
        return r.astype(np.float32)

    # B: y = clip(((<in1:h1> + s0)*t + s1)*z, -1, 1), t = min(z^2, imm2)
    _tb = minn(sq(Src0), C2)
    bodyB = minn(maxx(((Src1 + C0) * _tb + C1) * Src0, Zero - One), One)

    def refB(in0, in1, s0, s1, imm2):
        z = in0.astype(np.float32)
        t = np.minimum(z * z, np.float32(imm2))
        y = ((np.asarray(in1, np.float32) + np.float32(s0)) * t
             + np.float32(s1)) * z
        return np.clip(y, -1.0, 1.0).astype(np.float32)

    ops = []
    for name, body, ref in [("TANH9A_ANT", bodyA, refA),
                            ("TANH9B_ANT", bodyB, refB)]:
        existing = next((o for o in dve_ops.OPS if o.name == name), None)
        if existing is not None:
            ops.append(existing)
            continue
        spec = Spec(body=body, reference=ref)
        opcode = dve_ops._CUSTOM_DVE_ROW_BASE + len(dve_ops.OPS)
        assert opcode < 0x20
        shas = {}
        for ver in ("v3", "v4"):
            try:
                uops = lower(spec, ver=ver)
                shas[ver] = DveOpSpec(name=name, opcode=opcode, uops=uops,
                                      rd1_en=_has_src1(spec)).sha(ver)
            except Exception:
                pass  # ver not lowerable; only the ver in use matters
        op = dve_ops.DveOp(name, spec, subdim=False, uops_sha=shas)
        dve_ops.OPS.append(op)
        dve_ops.CUSTOM_DVE_SPECS[name] = spec
        dve_ops._SUB_OPCODE_FOR_NAME[name] = opcode
        ops.append(op)
    _TANH9 = tuple(ops)
    return _TANH9


def _dve_level(c):
    """Which level's tanh (if any) the DVE owns for chunk c.

    Rotating l = c%3 spreads the polynomial's error across levels; 3 of
    every 8 chunks stay fully on ACT to balance engine time (measured on
    HW: ACT tanh 0.69us/level-chunk, DVE pair 2.29us/level-chunk, plus
    the out-stage PSUM->SBUF copy ~0.22us/chunk on the DVE)."""
    if c % 8 in (2, 5, 7):
        return None
    return c % 3


def _build_program(use_bias: bool, reps: int = 1, use_dve: bool = True,
                   dma_once: bool = False):
    use_dve = use_dve and not use_bias
    if use_dve:
        tanh9a, tanh9b = _register_tanh9()
    nc = bacc.Bacc("TRN2", debug=False, target_bir_lowering=False,
                   num_devices=N_CORES)

    xg_d = nc.dram_tensor("xg", [128, G, BS], F16, kind="ExternalInput")
    w0_d = nc.dram_tensor("w0", [128, G, U], F16, kind="ExternalInput")
    weff_d = nc.dram_tensor("weff", [128, 2 * (L - 1), GU], F16,
                            kind="ExternalInput")
    wout_d = nc.dram_tensor("wout", [128, 2, OUT], F16, kind="ExternalInput")
    if use_bias:
        bias_d = nc.dram_tensor("bias", [128, 2 * L], F32, kind="ExternalInput")
    outt_d = nc.dram_tensor("outt", [OUT, BS], F32, kind="ExternalOutput")

    Tanh = mybir.ActivationFunctionType.Tanh
    C2K = 2 * CHUNK  # flat free size of a level tile (both mt halves)

    with tile.TileContext(nc) as tc:
        with tc.tile_pool(name="const", bufs=1) as cpool, \
             tc.tile_pool(name="xp", bufs=(17 if dma_once else 7)) as xpool, \
             tc.tile_pool(name="hp", bufs=4) as hpool, \
             tc.tile_pool(name="h1p", bufs=2) as h1pool, \
             tc.tile_pool(name="obp", bufs=2) as obpool, \
             tc.tile_pool(name="zp", bufs=3, space="PSUM") as zpool, \
             tc.tile_pool(name="op", bufs=2, space="PSUM") as opool:

            w0_t = cpool.tile([128, G, U], F16)
            nc.sync.dma_start(w0_t[:, :, :], w0_d[:, :, :])
            weff_t = cpool.tile([128, 2 * (L - 1), GU], F16)
            wout_t = cpool.tile([128, 2, OUT], F16)
            if use_bias:
                bias_t = cpool.tile([128, 2 * L], F32)
            if use_dve:
                a2c_t = cpool.tile([128, 1], F32)
                nc.gpsimd.memset(a2c_t[:, :], P9_A2)

            # trigger the ACT tanh table-set load early
            warm_in = cpool.tile([128, 1], F32)
            warm_out = cpool.tile([128, 1], F16)
            nc.gpsimd.memset(warm_in[:, :], 0.0)
            nc.scalar.activation(warm_out[:, :], warm_in[:, :], Tanh)

            xblocks = [(0, CHUNK), (CHUNK, CHUNK)]
            off = 2 * CHUNK
            while off < BS:
                sz = min(XBLK, BS - off)
                xblocks.append((off, sz))
                off += sz
            chunk_block = {}
            for bi, (s, sz) in enumerate(xblocks):
                for c in range(s // CHUNK, (s + sz) // CHUNK):
                    chunk_block[c] = bi

            xts_persist = {}
            for _rep in range(reps):
                xts = xts_persist if dma_once else {}
                hs = [{} for _ in range(L)]
                obs = {}

                def load_x(c):
                    bi = chunk_block[c]
                    if bi in xts:
                        return
                    s, sz = xblocks[bi]
                    t = xpool.tile([128, G, sz], F16, tag="x",
                                   name=f"xr{_rep}b{bi}",
                                   padded_shape=[128, G, XBLK])
                    if bi == 0 and _rep == 0:
                        nc.sync.dma_start(t[:, 0:2, :],
                                          xg_d[:, 0:2, s:s + sz])
                        nc.sync.dma_start(t[:, 2:4, :],
                                          xg_d[:, 2:4, s:s + sz])
                    else:
                        eng = nc.gpsimd if bi % 2 else nc.sync
                        eng.dma_start(t[:, :, :], xg_d[:, :, s:s + sz])
                    xts[bi] = t

                def level(c, l):
                    # z flat [128, 2*CHUNK]: halves at columns [0,CHUNK) and
                    # [CHUNK, 2*CHUNK)
                    z = zpool.tile([128, C2K], F32, tag="z",
                                   name=f"zr{_rep}c{c}l{l}")
                    if l == 0:
                        bi = chunk_block[c]
                        s, sz = xblocks[bi]
                        xoff = c * CHUNK - s
                        for pair in range(2):
                            for j in range(2):
                                g = 2 * pair + j
                                nc.tensor.matmul(
                                    z[64 * j:64 * (j + 1),
                                      pair * CHUNK:(pair + 1) * CHUNK],
                                    w0_t[:, g, :],
                                    xts[bi][:, g, xoff:xoff + CHUNK],
                                    start=True, stop=True)
                    else:
                        for mt in range(2):
                            for kt in range(2):
                                rhs = hs[l - 1][c][:,
                                                   kt * CHUNK:(kt + 1) * CHUNK]
                                nc.tensor.matmul(
                                    z[:, mt * CHUNK:(mt + 1) * CHUNK],
                                    weff_t[:, (l - 1) * 2 + kt,
                                           mt * 128:(mt + 1) * 128],
                                    rhs,
                                    start=(kt == 0), stop=(kt == 1))
                    hcur = hpool.tile([128, C2K], F16, tag=f"h{l}",
                                      name=f"hr{_rep}c{c}l{l}")
                    if use_bias:
                        for mt in range(2):
                            nc.scalar.activation(
                                hcur[:, mt * CHUNK:(mt + 1) * CHUNK],
                                z[:, mt * CHUNK:(mt + 1) * CHUNK], Tanh,
                                bias=bias_t[:, l * 2 + mt:l * 2 + mt + 1])
                    elif use_dve and _dve_level(c) == l:
                        h1t = h1pool.tile([128, C2K], F16, tag="h1",
                                          name=f"h1r{_rep}c{c}l{l}")
                        nc.vector._custom_dve(
                            tanh9a, out=h1t[:, :], in0=z[:, :],
                            in1=a2c_t[:, :],
                            s0=P9_CSQ, s1=P9_A4, imm2=P9_A3)
                        nc.vector._custom_dve(
                            tanh9b, out=hcur[:, :], in0=z[:, :],
                            in1=h1t[:, :],
                            s0=P9_A1, s1=P9_A0, imm2=P9_CSQ)
                    else:
                        nc.scalar.activation(hcur[:, :], z[:, :], Tanh)
                    hs[l][c] = hcur
                    if l > 0:
                        del hs[l - 1][c]

                def out_layer(c):
                    # 3 chunks share one PSUM tile, their [10, CHUNK] results
                    # at partitions {0,32,64} (bass allows matmul out base
                    # partitions 0/32/64 only) -> ONE DVE copy per 3 chunks
                    # (DVE time scales with free size only), then per-chunk
                    # DMAs on the gpsimd ring.
                    j = c % 3
                    sup = c // 3
                    if j == 0:
                        obs["po"] = opool.tile([74, CHUNK], F32, tag="po",
                                               name=f"por{_rep}s{sup}")
                    po = obs["po"]
                    for kt in range(2):
                        nc.tensor.matmul(po[32 * j:32 * j + OUT, :],
                                         wout_t[:, kt, :],
                                         hs[L - 1][c][:,
                                                      kt * CHUNK:(kt + 1) * CHUNK],
                                         start=(kt == 0), stop=(kt == 1))
                    del hs[L - 1][c]
                    if j == 2 or c == NCHUNK - 1:
                        ob = obpool.tile([74, CHUNK], F32, tag="ob",
                                         name=f"obr{_rep}s{sup}")
                        nc.vector.tensor_copy(ob[:, :], po[:, :])
                        del obs["po"]
                        for jj in range(j + 1):
                            cj = sup * 3 + jj
                            eng = (nc.sync if cj == NCHUNK - 1
                                   else nc.gpsimd)
                            eng.dma_start(
                                outt_d[:, cj * CHUNK:(cj + 1) * CHUNK],
                                ob[32 * jj:32 * jj + OUT, :])

                load_x(0)
                if _rep == 0:
                    nc.sync.dma_start(weff_t[:, 0:2, :], weff_d[:, 0:2, :])
                load_x(1)
                if _rep == 0:
                    nc.sync.dma_start(weff_t[:, 2:4, :], weff_d[:, 2:4, :])
                    nc.sync.dma_start(wout_t[:, :, :], wout_d[:, :, :])
                    if use_bias:
                        nc.sync.dma_start(bias_t[:, :], bias_d[:, :])
                for i in range(NCHUNK + L):
                    if i - L >= 0:
                        out_layer(i - L)
                    levels_t = [l for l in range(L - 1, -1, -1)
                                if 0 <= i - l < NCHUNK]
                    levels_t.sort(
                        key=lambda l: (0 if _dve_level(i - l) == l else 1,
                                       -l))
                    for l in levels_t:
                        level(i - l, l)
                    for ahead in (1, 2, 3, 4, 5):
                        if i + ahead < NCHUNK:
                            load_x(i + ahead)

    nc.compile()
    return nc


def _prepare_in_maps(x, idx, W, b, W_out):
    """Host-side prep: weight folding, layouts, shard + transpose + cast."""
    Weff = np.zeros((L - 1, D, GU), np.float32)
    for l in range(1, L):
        for g in range(G):
            np.add.at(Weff[l - 1, :, g * U:(g + 1) * U], idx[l, g], W[l, g])

    weff_dev = np.ascontiguousarray(
        Weff.reshape(L - 1, 2, 128, GU).transpose(2, 0, 1, 3)
        .reshape(128, 2 * (L - 1), GU)).astype(np.float16)
    w0_dev = np.ascontiguousarray(
        W[0].transpose(1, 0, 2)).astype(np.float16)       # [128, G, U]
    wout_dev = np.ascontiguousarray(
        W_out.reshape(2, 128, OUT).transpose(1, 0, 2)).astype(
        np.float16)
    idx0 = idx[0].reshape(-1)                             # [G*K]

    use_bias = bool(np.any(b != 0.0))
    bias_dev = np.ascontiguousarray(
        b.reshape(L, 2, 128).transpose(2, 0, 1).reshape(128, 2 * L)) \
        if use_bias else None

    in_maps = []
    for c in range(N_CORES):
        xs = x[c * BS:(c + 1) * BS]                       # [BS, 256]
        xt = xs.T.astype(np.float16)                      # [256, BS] contig
        xg = xt[idx0].reshape(G, 128, BS).transpose(1, 0, 2)
        m = {"xg": np.ascontiguousarray(xg),
             "w0": w0_dev, "weff": weff_dev, "wout": wout_dev}
        if use_bias:
            m["bias"] = bias_dev
        in_maps.append(m)
    return in_maps, use_bias


def kernel(x, idx, W, b, W_out, b_out):
    global LAST_RESULTS
    x = np.asarray(x, dtype=np.float32)
    idx = np.asarray(idx, dtype=np.int32)
    W = np.asarray(W, dtype=np.float32)
    b = np.asarray(b, dtype=np.float32)
    W_out = np.asarray(W_out, dtype=np.float32)
    b_out = np.asarray(b_out, dtype=np.float32)

    in_maps, use_bias = _prepare_in_maps(x, idx, W, b, W_out)

    nc = _PROG_CACHE.get(use_bias)
    if nc is None:
        nc = _PROG_CACHE[use_bias] = _build_program(use_bias)

    res = run_bass_kernel_spmd(nc, in_maps, list(range(N_CORES)),
                               trace=TRACE)
    LAST_RESULTS = res

    out = np.empty((B, OUT), np.float32)
    for c in range(N_CORES):
        out[c * BS:(c + 1) * BS] = res.results[c]["outt"].T
    if np.any(b_out != 0.0):
        out += b_out[None, :]
    return out


# revision 10
# speedup vs baseline: 1.1360x; 1.1360x over previous
"""Trainium2 Bass kernel for nn_DirectEncodingModel (gnn_message_passing).

Model (reference):
    h = x                                  # [B, 256]
    for l in 0..2:
        gathered = h[:, idx[l]]            # [B, 4, 128]
        z = einsum('bgk,gku->bgu', gathered, W[l]) + b[l]
        h = tanh(z).reshape(B, 256)
    out = h @ W_out + b_out                # [B, 10]

Host-side transforms (exact):
  * levels 1-2: fold the gather into dense per-level weights
        Weff[l][d, g*64+u] = sum_{k: idx[l,g,k]==d} W[l,g,k,u]
    so each level is h = tanh(h @ Weff[l]) — a dense [B,256]@[256,256]
    matmul.
  * level 0: host pre-gathers x per group (xg[g] = x[:, idx[0,g]]); the
    device runs one K=128, M=64 matmul per group with raw W[0,g]; the two
    M=64 halves of a pair occupy distinct PE column groups
    (tile_position via base partitions) and stream concurrently.

Device layout: activations transposed — [feature(partition), batch(free)],
fp16 matmuls with fp32 PSUM accumulation.

The bottleneck is the tanh: 12.58M elements/core, and the scalar (ACT)
engine alone runs ~96 us at 1 elem/lane/cycle.  v2 splits the tanh
between the ACT engine (table tanh) and the Vector engine (DVE) using two
runtime-registered custom DVE ops that evaluate a clamped degree-9 odd
minimax polynomial (max |err| vs tanh = 7.6e-3; end-to-end rel err
~8e-3 with the rotation schedule below, tolerance 2e-2):

    TANH9A: h1 = ((a4*t + a3)*t + a2)*t,        t = min(z^2, c^2)
    TANH9B: y  = clip(((h1 + a1)*t + a0)*z, -1, 1)

Each custom op processes 1 elem/lane/cycle on the DVE (multi-uop single
instruction), so a DVE tanh pair costs ~2.3 us/level-chunk vs ACT 0.69
(both HW-measured) — the balanced split is 5/8 of chunks' rotating level
on the DVE (see _dve_level).  The PE (fp16 matmul measured at ~0.21
ns/col, 2x the cost model; L0 uses M=64 column-pair concurrency) runs
~47 us, under the ~53 us tanh-engine wall.

Out layer: 3 chunks' [10,512] results pack into one PSUM tile at
partitions {0,32,64} via tile_position, one DVE copy per 3 chunks, DMA
out from SBUF on the gpsimd ring.

x DMA (16.8 MB/core fp16, pre-gathered) alternates between the sync and
gpsimd DGE rings — a single ring was rate-limiting (~19 us of exposed
DMA in the marginal-rep measurement) — with a 5-block prefetch horizon.
Per tick the DVE-owned level's matmuls are emitted first so the DVE feed
never queues behind PE write-after-read stalls on ACT-owned levels.

Software pipeline (skewed emission out(i-3) | L2(i-2) | L1(i-1) | L0(i))
is unchanged from v1.  Measured: ~62 us/rep marginal (paired-slope, 150
pairs), rel err 7.4e-3; v1 baseline was ~91 us.  Sharding: pure batch
data parallelism across 8 cores; weights replicated.
"""

import numpy as np

import concourse.mybir as mybir
import concourse.bacc as bacc
import concourse.tile as tile
from concourse.bass_utils import run_bass_kernel_spmd

F16 = mybir.dt.float16
F32 = mybir.dt.float32

N_CORES = 8
B, D, L, G, K, U, OUT = 131072, 256, 3, 4, 128, 64, 10
GU = G * U  # 256
BS = B // N_CORES  # 16384 per core

CHUNK = 512           # batch columns per level-computation (one PSUM slot)
NCHUNK = BS // CHUNK  # 32
XBLK = 1024           # batch columns per x DMA
OBLK = 1024           # batch columns per output DMA

# test-harness hooks (harness never touches these; defaults are production)
TRACE = False
LAST_RESULTS = None

_PROG_CACHE = {}

# ---------------------------------------------------------------------------
# Custom DVE tanh (degree-9 odd minimax, ramp + output clamp form)
# y = clip(z*(a0 + t*(a1 + t*(a2 + t*(a3 + t*a4)))), -1, 1), t = min(z^2, c^2)
P9_A0 = 0.9800317416370952
P9_A1 = -0.26106888154251784
P9_A2 = 0.05544444997442462
P9_A3 = -0.006477015514642958
P9_A4 = 0.0003014239240568509
P9_C = 2.969877822220705
P9_CSQ = P9_C * P9_C

_TANH9 = None  # (opA, opB) after registration


def _register_tanh9():
    """Register the two custom DVE ops with concourse.dve_ops at runtime
    (the documented extension point is the module-level OPS list; we can't
    edit the read-only repo, so we append in-process before any compile)."""
    global _TANH9
    if _TANH9 is not None:
        return _TANH9
    import concourse.dve_ops as dve_ops
    from concourse.dve_spec import (Spec, Src0, Src1, C0, C1, C2, C3, Zero,
                                    One, maxx, minn, sq, lower,
                                    _spill_c3_to_src1, _has_src1)
    from concourse.dve_uop import DveOpSpec

    # A: h1 = ((s1*t + imm2)*t + <in1:[P,1]>)*t, t = min(z^2, s0)
    _t = minn(sq(Src0), C0)
    bodyA = _spill_c3_to_src1(((C1 * _t + C2) * _t + C3) * _t)

    def refA(in0, in1, s0, s1, imm2):
        t = np.minimum(in0.astype(np.float32) ** 2, np.float32(s0))
        r = ((np.float32(s1) * t + np.float32(imm2)) * t
             + np.asarray(in1, np.float32)) * t
        return r.astype(np.float32)

    # B: y = clip(((<in1:h1> + s0)*t + s1)*z, -1, 1), t = min(z^2, imm2)
    _tb = minn(sq(Src0), C2)
    bodyB = minn(maxx(((Src1 + C0) * _tb + C1) * Src0, Zero - One), One)

    def refB(in0, in1, s0, s1, imm2):
        z = in0.astype(np.float32)
        t = np.minimum(z * z, np.float32(imm2))
        y = ((np.asarray(in1, np.float32) + np.float32(s0)) * t
             + np.float32(s1)) * z
        return np.clip(y, -1.0, 1.0).astype(np.float32)

    ops = []
    for name, body, ref in [("TANH9A_ANT", bodyA, refA),
                            ("TANH9B_ANT", bodyB, refB)]:
        existing = next((o for o in dve_ops.OPS if o.name == name), None)
        if existing is not None:
            ops.append(existing)
            continue
        spec = Spec(body=body, reference=ref)
        opcode = dve_ops._CUSTOM_DVE_ROW_BASE + len(dve_ops.OPS)
        assert opcode < 0x20
        shas = {}
        for ver in ("v3", "v4"):
            try:
                uops = lower(spec, ver=ver)
                shas[ver] = DveOpSpec(name=name, opcode=opcode, uops=uops,
                                      rd1_en=_has_src1(spec)).sha(ver)
            except Exception:
                pass  # ver not lowerable; only the ver in use matters
        op = dve_ops.DveOp(name, spec, subdim=False, uops_sha=shas)
        dve_ops.OPS.append(op)
        dve_ops.CUSTOM_DVE_SPECS[name] = spec
        dve_ops._SUB_OPCODE_FOR_NAME[name] = opcode
        ops.append(op)
    _TANH9 = tuple(ops)
    return _TANH9


def _dve_level(c):
    """Which level's tanh (if any) the DVE owns for chunk c.

    Rotating l = c%3 spreads the polynomial's error across levels; 3 of
    every 8 chunks stay fully on ACT to balance engine time (measured on
    HW: ACT tanh 0.69us/level-chunk, DVE pair 2.29us/level-chunk, plus
    the out-stage PSUM->SBUF copy ~0.22us/chunk on the DVE)."""
    if c % 8 in (2, 5, 7):
        return None
    return c % 3


def _build_program(use_bias: bool, reps: int = 1, use_dve: bool = True,
                   dma_once: bool = False):
    use_dve = use_dve and not use_bias
    if use_dve:
        tanh9a, tanh9b = _register_tanh9()
    nc = bacc.Bacc("TRN2", debug=False, target_bir_lowering=False,
                   num_devices=N_CORES)

    xg_d = nc.dram_tensor("xg", [128, G, BS], F16, kind="ExternalInput")
    w0_d = nc.dram_tensor("w0", [128, G, U], F16, kind="ExternalInput")
    weff_d = nc.dram_tensor("weff", [128, 2 * (L - 1), GU], F16,
                            kind="ExternalInput")
    wout_d = nc.dram_tensor("wout", [128, 2, OUT], F16, kind="ExternalInput")
    if use_bias:
        bias_d = nc.dram_tensor("bias", [128, 2 * L], F32, kind="ExternalInput")
    outt_d = nc.dram_tensor("outt", [OUT, BS], F32, kind="ExternalOutput")

    Tanh = mybir.ActivationFunctionType.Tanh
    C2K = 2 * CHUNK  # flat free size of a level tile (both mt halves)

    with tile.TileContext(nc) as tc:
        with tc.tile_pool(name="const", bufs=1) as cpool, \
             tc.tile_pool(name="xp", bufs=(17 if dma_once else 10)) as xpool, \
             tc.tile_pool(name="hp", bufs=4) as hpool, \
             tc.tile_pool(name="h1p", bufs=2) as h1pool, \
             tc.tile_pool(name="obp", bufs=2) as obpool, \
             tc.tile_pool(name="zp", bufs=3, space="PSUM") as zpool, \
             tc.tile_pool(name="op", bufs=2, space="PSUM") as opool:

            w0_t = cpool.tile([128, G, U], F16)
            nc.sync.dma_start(w0_t[:, :, :], w0_d[:, :, :])
            weff_t = cpool.tile([128, 2 * (L - 1), GU], F16)
            wout_t = cpool.tile([128, 2, OUT], F16)
            if use_bias:
                bias_t = cpool.tile([128, 2 * L], F32)
            if use_dve:
                a2c_t = cpool.tile([128, 1], F32)
                nc.gpsimd.memset(a2c_t[:, :], P9_A2)

            # trigger the ACT tanh table-set load early
            warm_in = cpool.tile([128, 1], F32)
            warm_out = cpool.tile([128, 1], F16)
            nc.gpsimd.memset(warm_in[:, :], 0.0)
            nc.scalar.activation(warm_out[:, :], warm_in[:, :], Tanh)

            xblocks = [(0, CHUNK), (CHUNK, CHUNK)]
            off = 2 * CHUNK
            while off < BS:
                sz = min(XBLK, BS - off)
                xblocks.append((off, sz))
                off += sz
            chunk_block = {}
            for bi, (s, sz) in enumerate(xblocks):
                for c in range(s // CHUNK, (s + sz) // CHUNK):
                    chunk_block[c] = bi

            xts_persist = {}
            for _rep in range(reps):
                xts = xts_persist if dma_once else {}
                hs = [{} for _ in range(L)]
                obs = {}

                def load_x(c):
                    bi = chunk_block[c]
                    if bi in xts:
                        return
                    s, sz = xblocks[bi]
                    t = xpool.tile([128, G, sz], F16, tag="x",
                                   name=f"xr{_rep}b{bi}",
                                   padded_shape=[128, G, XBLK])
                    if bi == 0 and _rep == 0:
                        nc.sync.dma_start(t[:, 0:2, :],
                                          xg_d[:, 0:2, s:s + sz])
                        nc.sync.dma_start(t[:, 2:4, :],
                                          xg_d[:, 2:4, s:s + sz])
                    else:
                        eng = nc.gpsimd if bi % 2 else nc.sync
                        eng.dma_start(t[:, :, :], xg_d[:, :, s:s + sz])
                    xts[bi] = t

                def level(c, l):
                    # z flat [128, 2*CHUNK]: halves at columns [0,CHUNK) and
                    # [CHUNK, 2*CHUNK)
                    z = zpool.tile([128, C2K], F32, tag="z",
                                   name=f"zr{_rep}c{c}l{l}")
                    if l == 0:
                        bi = chunk_block[c]
                        s, sz = xblocks[bi]
                        xoff = c * CHUNK - s
                        for pair in range(2):
                            for j in range(2):
                                g = 2 * pair + j
                                nc.tensor.matmul(
                                    z[64 * j:64 * (j + 1),
                                      pair * CHUNK:(pair + 1) * CHUNK],
                                    w0_t[:, g, :],
                                    xts[bi][:, g, xoff:xoff + CHUNK],
                                    start=True, stop=True)
                    else:
                        for mt in range(2):
                            for kt in range(2):
                                rhs = hs[l - 1][c][:,
                                                   kt * CHUNK:(kt + 1) * CHUNK]
                                nc.tensor.matmul(
                                    z[:, mt * CHUNK:(mt + 1) * CHUNK],
                                    weff_t[:, (l - 1) * 2 + kt,
                                           mt * 128:(mt + 1) * 128],
                                    rhs,
                                    start=(kt == 0), stop=(kt == 1))
                    hcur = hpool.tile([128, C2K], F16, tag=f"h{l}",
                                      name=f"hr{_rep}c{c}l{l}")
                    if use_bias:
                        for mt in range(2):
                            nc.scalar.activation(
                                hcur[:, mt * CHUNK:(mt + 1) * CHUNK],
                                z[:, mt * CHUNK:(mt + 1) * CHUNK], Tanh,
                                bias=bias_t[:, l * 2 + mt:l * 2 + mt + 1])
                    elif use_dve and _dve_level(c) == l:
                        h1t = h1pool.tile([128, C2K], F16, tag="h1",
                                          name=f"h1r{_rep}c{c}l{l}")
                        nc.vector._custom_dve(
                            tanh9a, out=h1t[:, :], in0=z[:, :],
                            in1=a2c_t[:, :],
                            s0=P9_CSQ, s1=P9_A4, imm2=P9_A3)
                        nc.vector._custom_dve(
                            tanh9b, out=hcur[:, :], in0=z[:, :],
                            in1=h1t[:, :],
                            s0=P9_A1, s1=P9_A0, imm2=P9_CSQ)
                    else:
                        nc.scalar.activation(hcur[:, :], z[:, :], Tanh)
                    hs[l][c] = hcur
                    if l > 0:
                        del hs[l - 1][c]

                def out_layer(c):
                    # 3 chunks share one PSUM tile, their [10, CHUNK] results
                    # at partitions {0,32,64} (bass allows matmul out base
                    # partitions 0/32/64 only) -> ONE DVE copy per 3 chunks
                    # (DVE time scales with free size only), then per-chunk
                    # DMAs on the gpsimd ring.
                    j = c % 3
                    sup = c // 3
                    if j == 0:
                        obs["po"] = opool.tile([74, CHUNK], F32, tag="po",
                                               name=f"por{_rep}s{sup}")
                    po = obs["po"]
                    for kt in range(2):
                        nc.tensor.matmul(po[32 * j:32 * j + OUT, :],
                                         wout_t[:, kt, :],
                                         hs[L - 1][c][:,
                                                      kt * CHUNK:(kt + 1) * CHUNK],
                                         start=(kt == 0), stop=(kt == 1))
                    del hs[L - 1][c]
                    if j == 2 or c == NCHUNK - 1:
                        ob = obpool.tile([74, CHUNK], F32, tag="ob",
                                         name=f"obr{_rep}s{sup}")
                        nc.vector.tensor_copy(ob[:, :], po[:, :])
                        del obs["po"]
                        for jj in range(j + 1):
                            cj = sup * 3 + jj
                            eng = (nc.sync if cj == NCHUNK - 1
                                   else nc.gpsimd)
                            eng.dma_start(
                                outt_d[:, cj * CHUNK:(cj + 1) * CHUNK],
                                ob[32 * jj:32 * jj + OUT, :])

                load_x(0)
                if _rep == 0:
                    nc.sync.dma_start(weff_t[:, 0:2, :], weff_d[:, 0:2, :])
                load_x(1)
                if _rep == 0:
                    nc.sync.dma_start(weff_t[:, 2:4, :], weff_d[:, 2:4, :])
                    nc.sync.dma_start(wout_t[:, :, :], wout_d[:, :, :])
                    if use_bias:
                        nc.sync.dma_start(bias_t[:, :], bias_d[:, :])
                for i in range(NCHUNK + L):
                    for ahead in (1, 2, 3, 4, 5, 6, 7, 8):
                        if i + ahead < NCHUNK:
                            load_x(i + ahead)
                    if i - L >= 0:
                        out_layer(i - L)
                    levels_t = [l for l in range(L - 1, -1, -1)
                                if 0 <= i - l < NCHUNK]
                    levels_t.sort(
                        key=lambda l: (0 if _dve_level(i - l) == l else 1,
                                       -l))
                    for l in levels_t:
                        level(i - l, l)

    nc.compile()
    return nc


def _prepare_in_maps(x, idx, W, b, W_out):
    """Host-side prep: weight folding, layouts, shard + transpose + cast."""
    Weff = np.zeros((L - 1, D, GU), np.float32)
    for l in range(1, L):
        for g in range(G):
            np.add.at(Weff[l - 1, :, g * U:(g + 1) * U], idx[l, g], W[l, g])

    weff_dev = np.ascontiguousarray(
        Weff.reshape(L - 1, 2, 128, GU).transpose(2, 0, 1, 3)
        .reshape(128, 2 * (L - 1), GU)).astype(np.float16)
    w0_dev = np.ascontiguousarray(
        W[0].transpose(1, 0, 2)).astype(np.float16)       # [128, G, U]
    wout_dev = np.ascontiguousarray(
        W_out.reshape(2, 128, OUT).transpose(1, 0, 2)).astype(
        np.float16)
    idx0 = idx[0].reshape(-1)                             # [G*K]

    use_bias = bool(np.any(b != 0.0))
    bias_dev = np.ascontiguousarray(
        b.reshape(L, 2, 128).transpose(2, 0, 1).reshape(128, 2 * L)) \
        if use_bias else None

    in_maps = []
    for c in range(N_CORES):
        xs = x[c * BS:(c + 1) * BS]                       # [BS, 256]
        xt = xs.T.astype(np.float16)                      # [256, BS] contig
        xg = xt[idx0].reshape(G, 128, BS).transpose(1, 0, 2)
        m = {"xg": np.ascontiguousarray(xg),
             "w0": w0_dev, "weff": weff_dev, "wout": wout_dev}
        if use_bias:
            m["bias"] = bias_dev
        in_maps.append(m)
    return in_maps, use_bias


def kernel(x, idx, W, b, W_out, b_out):
    global LAST_RESULTS
    x = np.asarray(x, dtype=np.float32)
    idx = np.asarray(idx, dtype=np.int32)
    W = np.asarray(W, dtype=np.float32)
    b = np.asarray(b, dtype=np.float32)
    W_out = np.asarray(W_out, dtype=np.float32)
    b_out = np.asarray(b_out, dtype=np.float32)

    in_maps, use_bias = _prepare_in_maps(x, idx, W, b, W_out)

    nc = _PROG_CACHE.get(use_bias)
    if nc is None:
        nc = _PROG_CACHE[use_bias] = _build_program(use_bias)

    res = run_bass_kernel_spmd(nc, in_maps, list(range(N_CORES)),
                               trace=TRACE)
    LAST_RESULTS = res

    out = np.empty((B, OUT), np.float32)
    for c in range(N_CORES):
        out[c * BS:(c + 1) * BS] = res.results[c]["outt"].T
    if np.any(b_out != 0.0):
        out += b_out[None, :]
    return out
